# revision 7
# baseline (speedup 1.0000x reference)
"""Causal self-attention on 8 TRN2 NeuronCores — collective-free version.

Sharding: rank r = 2*b + g (b = batch 0..3, g = head-group 0..1; 8 heads per
group). Per core: QKV projection for its head-group, causal attention with
scores kept transposed — (tk, tq) tiles so the softmax denominator comes from
a ones-column folded into V — then a PARTIAL output projection: each core
contracts its local 512 attention channels against w_out[:, group] producing
a full (1024, T) partial y; the host adds the two partials per batch. This
removes the AllGather entirely (collectives cost 60-200us of exposed wall
time in this runtime, varying with network load).

Everything flows in bf16 (PE rate is identical to f32r for N>=256 and bf16
lifts the N>=256 restriction so diagonal score/AV tiles shrink to their exact
column ranges). PSUM accumulation stays f32.

The AV contraction runs TRANSPOSED: per 128-query subtile, out[q, 65] +=
e_tile[keys, queries].T @ v_aug[keys, 65] — the 65-wide moving operand makes
each key tile ~2.5x cheaper on PE than the (65, 512) orientation (measured),
and the softmax denominator lands as a per-partition scalar so normalization
is one reciprocal + one tensor_scalar multiply. The four query-subtile
accumulators share one PSUM bank, so AV matmuls never use start=True (a start
zeroes the whole 2KB bank) — the bank is memset once per head instead.
Normalized head-pairs collect in a (128 queries x 128 channels) staging tile
and one XBAR transposing DMA drops them into the outproj layout.

Scheduling: the QKV projection is software-pipelined INTO the attention
stream. Attention query-quarter n only needs QKV t-quarters <= n, so QKV
chains for quarter tq+1 are emitted as PE fillers between attention groups of
quarter tq. Quarters 0-2 are PE-bound (chains fill all bubbles), quarter 3 is
exp(ACT)-bound, so ALL output-projection units are deferred into quarter 3's
PE bubbles. QKV chains and outproj units share one double-buffered PSUM pool
(they are never concurrently active), which keeps chain matmuls from stalling
on the previous chain's PSUM->SBUF copy.
"""
import numpy as np
import ml_dtypes

import concourse.bass as bass
import concourse.mybir as mybir
import concourse.tile as tile
from concourse import bacc
from concourse.bass_utils import run_bass_kernel_spmd

F32 = mybir.dt.float32
BF16 = mybir.dt.bfloat16
EXP = mybir.ActivationFunctionType.Exp
NP_BF16 = ml_dtypes.bfloat16

B, T, C, H, HD = 4, 2048, 1024, 16, 64
G, HG, CG = 2, 8, 512          # head groups, heads/group, channels/group
NCORES = 8
NEG = -1.0e30

_cache = {}


def _build(unroll=1):
    nc = bacc.Bacc("TRN2", target_bir_lowering=False, debug=False,
                   num_devices=NCORES)

    xT = nc.dram_tensor("xT", [C, T], BF16, kind="ExternalInput")
    w_qT = nc.dram_tensor("w_qT", [C, CG], BF16, kind="ExternalInput")
    w_kT = nc.dram_tensor("w_kT", [C, CG], BF16, kind="ExternalInput")
    w_vT = nc.dram_tensor("w_vT", [C, CG], BF16, kind="ExternalInput")
    w_oT = nc.dram_tensor("w_oT", [CG, C], BF16, kind="ExternalInput")
    maskadd = nc.dram_tensor("maskadd", [128, 128], F32, kind="ExternalInput")
    y_part = nc.dram_tensor("y_part", [C, T], BF16, kind="ExternalOutput")

    with tile.TileContext(nc) as tc:
      for _it in range(unroll):
        with tc.tile_pool(name="attn_data", bufs=1) as p_data, \
             tc.tile_pool(name="consts", bufs=1) as p_const, \
             tc.tile_pool(name="xq", bufs=2) as p_x, \
             tc.tile_pool(name="wqkv", bufs=1) as p_w, \
             tc.tile_pool(name="ps_s", bufs=2, space="PSUM") as p_s, \
             tc.tile_pool(name="ps_o", bufs=2, space="PSUM") as p_o, \
             tc.tile_pool(name="ps_fy", bufs=2, space="PSUM") as p_fy, \
             tc.tile_pool(name="expS", bufs=5) as p_e, \
             tc.tile_pool(name="small", bufs=2) as p_sm, \
             tc.tile_pool(name="pair", bufs=8) as p_pair, \
             tc.tile_pool(name="ysb", bufs=2) as p_ysb:
            qT = p_data.tile([128, 4, T], BF16, tag="qT")    # (ch%128, ch//128, t)
            kT = p_data.tile([128, 4, T], BF16, tag="kT")
            v_aug = p_data.tile([128, 16, HG, HD + 1], BF16, tag="v")
            attn_sb = p_data.tile([128, 4, T], BF16, tag="attn")
            wo_all = p_data.tile([128, 4, C], BF16, tag="wo")
            wk_all = p_w.tile([128, 8, CG], BF16, tag="wk_all")
            wq_all = p_w.tile([128, 8, CG], BF16, tag="wq_all")
            vstrip = p_w.tile([128, 8, CG], BF16, tag="vstrip")
            masks = p_const.tile([128, 128], F32, tag="masks")

            xT_r = xT[:].rearrange("(ct p) t -> p ct t", p=128)     # (128, 8, T)
            wq_r = w_qT[:].rearrange("(ct p) m -> p ct m", p=128)   # (128, 8, CG)
            wk_r = w_kT[:].rearrange("(ct p) m -> p ct m", p=128)
            wv_r = w_vT[:].rearrange("(ct p) m -> p ct m", p=128)
            wo_r = w_oT[:].rearrange("(ct p) m -> p ct m", p=128)   # (128, 4, C)

            # ---- startup DMAs ----
            nc.sync.dma_start(out=wk_all[:], in_=wk_r)
            nc.sync.dma_start(out=masks[:], in_=maskadd[:])
            nc.sync.dma_start(out=wq_all[:], in_=wq_r)
            xq_tiles = {}
            for tq in (0, 1):
                xq = p_x.tile([128, 8, 512], BF16, tag="xq")
                xq_tiles[tq] = xq
                for ct in range(8):
                    nc.sync.dma_start(out=xq[:, ct, :],
                                      in_=xT_r[:, ct, tq * 512:(tq + 1) * 512])
            nc.sync.dma_start(out=vstrip[:], in_=wv_r)
            ones_bf = p_const.tile([128, 1], BF16, tag="ones_bf")
            nc.vector.memset(ones_bf[:], 1.0)
            nc.vector.tensor_copy(
                out=v_aug[:, :, :, HD:HD + 1],
                in_=ones_bf[:].to_broadcast([128, 16, HG, 1]))

            # ---- QKV chain emitters (each: 8 PE matmuls + 1 DVE copy) ----
            def qkv_chain(tq, kind, j):
                t0 = tq * 512
                xq = xq_tiles[tq]
                ps = p_fy.tile([128, 512], F32, tag="fy")
                if kind == "V":
                    for ct in range(8):
                        nc.tensor.matmul(
                            ps[:], xq[:, ct, j * 128:(j + 1) * 128],
                            vstrip[:, ct, :],
                            start=(ct == 0), stop=(ct == 7))
                    m = tq * 4 + j
                    nc.vector.tensor_copy(
                        out=v_aug[:, m, :, 0:HD],
                        in_=ps[:].rearrange("p (h d) -> p h d", h=HG))
                else:
                    dest, wsrc = ((kT, wk_all) if kind == "K"
                                  else (qT, wq_all))
                    for ct in range(8):
                        nc.tensor.matmul(
                            ps[:], wsrc[:, ct, j * 128:(j + 1) * 128],
                            xq[:, ct, :],
                            start=(ct == 0), stop=(ct == 7))
                    nc.vector.tensor_copy(
                        out=dest[:, j, t0:t0 + 512], in_=ps[:])

            # Only the kt=0 K and Q chains run before the first attention
            # group; the rest of quarter 0's chains interleave into the first
            # groups (pops happen BEFORE each group's emission, and the
            # 2-per-group early schedule keeps every chain ahead of the
            # scores/AV that read it in the in-order PE stream)
            qkv_chain(0, "K", 0)
            qkv_chain(0, "Q", 0)

            # ---- attention group stream with fused fillers ----
            import collections as _c
            fillers = _c.deque()
            op_fillers = _c.deque()
            for kind, j in (("K", 1), ("Q", 1), ("V", 0), ("V", 1),
                            ("V", 2), ("V", 3), ("K", 2), ("Q", 2),
                            ("K", 3), ("Q", 3)):
                fillers.append(lambda kind=kind, j=j: qkv_chain(0, kind, j))

            def _enqueue_qkv(tq):
                for kind in ("K", "Q", "V"):
                    for j in range(4):
                        fillers.append(
                            lambda tq=tq, kind=kind, j=j: qkv_chain(tq, kind, j))

            def _enqueue_outproj(n):
                # out-projection units go to a separate deque popped only
                # during quarter 3 (and the tail): quarters 0-2 are PE-bound
                # (QKV chains fill their bubbles), while quarter 3's exp
                # stream leaves PE idle ~20us — that's where these belong
                for co in range(8):
                    y_ps = p_fy.tile([128, 512], F32, tag="fy")
                    y_sb = p_ysb.tile([128, 512], BF16, tag="ysb")

                    def t1(co=co, n=n, y_ps=y_ps):
                        for ci in range(2):
                            nc.tensor.matmul(
                                y_ps[:], wo_all[:, ci, co * 128:(co + 1) * 128],
                                attn_sb[:, ci, n * 512:(n + 1) * 512],
                                start=(ci == 0), stop=False)

                    def t2(co=co, n=n, y_ps=y_ps, y_sb=y_sb):
                        for ci in range(2, 4):
                            nc.tensor.matmul(
                                y_ps[:], wo_all[:, ci, co * 128:(co + 1) * 128],
                                attn_sb[:, ci, n * 512:(n + 1) * 512],
                                start=False, stop=(ci == 3))
                        nc.vector.tensor_copy(out=y_sb[:], in_=y_ps[:])
                        nc.sync.dma_start(
                            out=y_part[co * 128:(co + 1) * 128,
                                       n * 512:(n + 1) * 512],
                            in_=y_sb[:])
                    op_fillers.append(t1)
                    op_fillers.append(t2)

            o_ps_cur = {}
            pair_tiles = {}

            def _retire(g):
                # Transposed AV: per 128-query subtile q, accumulate
                # out[q_part, 65] += e_tile[128k, 128q].T @ v_aug[128k, 65].
                # Out free size is 65 (bf16, no N>=256 restriction), ~2.5x
                # cheaper per key tile than the [65, 512] orientation, and the
                # softmax denominator lands as a per-partition scalar.
                n, h, grp, ngrp = g
                kt, po = h // 2, (h % 2) * 64
                o_ps = o_ps_cur[(n, h)]["o"]
                e_sb = o_ps_cur[(n, h)]["e"][grp]
                # start=False always: a start would zero the whole 2KB PSUM
                # bank, clobbering the other query-subtiles' accumulators
                # (ZERO_REGION_SIZE=2048). The bank is memset once per head.
                for jj in range(2):
                    m = grp * 2 + jj
                    o4 = m - 4 * n
                    for q in range(4):
                        if o4 > q:
                            continue
                        nc.tensor.matmul(
                            o_ps[:, q, 0:HD + 1],
                            e_sb[:, jj, q * 128:(q + 1) * 128],
                            v_aug[:, m, h, :],
                            start=False, stop=(m == 4 * n + q),
                            skip_group_check=True)
                if grp == ngrp - 1:
                    # normalize into a head-PAIR staging tile (128 queries x
                    # 128 channels); when the odd head completes, one
                    # transposing DMA (XBAR needs 128-col tiles) lands both
                    # heads' channels in the outproj staging layout
                    for q in range(4):
                        rzq = p_sm.tile([128, 1], F32, tag="rzq")
                        nc.vector.reciprocal(rzq[:], o_ps[:, q, HD:HD + 1])
                        if h % 2 == 0:
                            pairT = p_pair.tile([128, 128], BF16, tag="pairT")
                            pair_tiles[(n, kt, q)] = pairT
                        else:
                            pairT = pair_tiles.pop((n, kt, q))
                        nc.vector.tensor_scalar_mul(
                            pairT[:, po:po + HD], o_ps[:, q, 0:HD], rzq[:])
                        if h % 2 == 1:
                            nc.sync.dma_start(
                                out=attn_sb[:, kt,
                                            n * 512 + q * 128:
                                            n * 512 + (q + 1) * 128],
                                in_=pairT[:], transpose=True)
                    del o_ps_cur[(n, h)]
                    if h == HG - 1:
                        _enqueue_outproj(n)

            pend = _c.deque()
            gi = 0
            for n in range(4):
                ngrp = 2 * n + 2
                # prefetch x quarter n+2 (reuses buffer of quarter n, whose
                # QKV chains are already done) and enqueue next QKV quarter
                if n + 2 <= 3:
                    xq = p_x.tile([128, 8, 512], BF16, tag="xq")
                    xq_tiles[n + 2] = xq
                    for ct in range(8):
                        nc.sync.dma_start(
                            out=xq[:, ct, :],
                            in_=xT_r[:, ct, (n + 2) * 512:(n + 3) * 512])
                if n == 0:
                    nc.sync.dma_start(out=wo_all[:], in_=wo_r)
                if n + 1 <= 3:
                    _enqueue_qkv(n + 1)
                for h in range(HG):
                    for grp in range(ngrp):
                        # pop fillers BEFORE emitting the group so chains a
                        # deferred retire depends on precede it in PE order;
                        # 2/group during the first groups of the run
                        npop = 2 if (n == 0 and gi < 5) else 1
                        for _ in range(npop):
                            if fillers:
                                fillers.popleft()()
                            elif n == 3 and op_fillers:
                                op_fillers.popleft()()
                        if grp == 0:
                            o_ps = p_o.tile([128, 4, 128], F32, tag="o",
                                            name=f"ops_{n}_{h}")
                            nc.vector.memset(o_ps[:, :, 0:HD + 1], 0.0)
                            o_ps_cur[(n, h)] = {"o": o_ps, "e": {}}
                        s_ps = p_s.tile([128, 2, 512], F32, tag="s")
                        e_sb = p_e.tile([128, 2, 512], BF16, tag="e")
                        o_ps_cur[(n, h)]["e"][grp] = e_sb
                        for jj in range(2):
                            m = grp * 2 + jj
                            o4 = m - 4 * n
                            j0 = 0 if o4 < 0 else 128 * o4
                            po, kt = (h % 2) * 64, h // 2
                            nc.tensor.matmul(
                                s_ps[:, jj, j0:512],
                                kT[po:po + 64, kt, m * 128:(m + 1) * 128],
                                qT[po:po + 64, kt,
                                   n * 512 + j0:(n + 1) * 512],
                                start=True, stop=True)
                            if o4 >= 0:
                                nc.vector.tensor_add(
                                    s_ps[:, jj, 128 * o4:128 * (o4 + 1)],
                                    s_ps[:, jj, 128 * o4:128 * (o4 + 1)],
                                    masks[:])
                        if grp == ngrp - 1:
                            # diagonal group (o4 = 2, 3): exp only the
                            # column ranges the matmuls wrote
                            nc.scalar.activation(
                                e_sb[:, 0, 256:512], s_ps[:, 0, 256:512],
                                EXP, scale=0.125)
                            nc.scalar.activation(
                                e_sb[:, 1, 384:512], s_ps[:, 1, 384:512],
                                EXP, scale=0.125)
                        else:
                            nc.scalar.activation(e_sb[:], s_ps[:], EXP,
                                                 scale=0.125)
                        pend.append((n, h, grp, ngrp))
                        if len(pend) > 3:
                            _retire(pend.popleft())
                        gi += 1

            while pend:
                _retire(pend.popleft())
            while fillers:
                fillers.popleft()()
            while op_fillers:
                op_fillers.popleft()()

    nc.compile()
    return nc


def _mask_np():
    i = np.arange(128, dtype=np.int64)[:, None]
    j = np.arange(128, dtype=np.int64)[None, :]
    return np.where(i > j, np.float32(NEG), np.float32(0.0)).astype(np.float32)


def _in_maps(x, w_qkv, w_out):
    mask = _mask_np()
    xTs = {b: np.ascontiguousarray(x[b].T).astype(NP_BF16) for b in range(B)}
    wT = {}
    for g in range(G):
        wT[("q", g)] = np.ascontiguousarray(
            w_qkv[g * CG:(g + 1) * CG, :].T).astype(NP_BF16)
        wT[("k", g)] = np.ascontiguousarray(
            w_qkv[C + g * CG:C + (g + 1) * CG, :].T).astype(NP_BF16)
        wT[("v", g)] = np.ascontiguousarray(
            w_qkv[2 * C + g * CG:2 * C + (g + 1) * CG, :].T).astype(NP_BF16)
        wT[("o", g)] = np.ascontiguousarray(
            w_out[:, g * CG:(g + 1) * CG].T).astype(NP_BF16)
    maps = []
    for r in range(NCORES):
        b, g = r // 2, r % 2
        maps.append({
            "xT": xTs[b],
            "w_qT": wT[("q", g)],
            "w_kT": wT[("k", g)],
            "w_vT": wT[("v", g)],
            "w_oT": wT[("o", g)],
            "maskadd": mask,
        })
    return maps


def _run(x, w_qkv, w_out, trace=False):
    if "nc" not in _cache:
        _cache["nc"] = _build()
    res = run_bass_kernel_spmd(_cache["nc"], _in_maps(x, w_qkv, w_out),
                               list(range(NCORES)), trace=trace)
    y = np.empty((B, T, C), np.float32)
    for b in range(B):
        p0 = res.results[2 * b]["y_part"].astype(np.float32)
        p1 = res.results[2 * b + 1]["y_part"].astype(np.float32)
        y[b] = (p0 + p1).T
    return y, res


def kernel(x, w_qkv, w_out):
    x = np.asarray(x, dtype=np.float32)
    w_qkv = np.asarray(w_qkv, dtype=np.float32)
    w_out = np.asarray(w_out, dtype=np.float32)
    y, _ = _run(x, w_qkv, w_out)
    return y
